# revision 17
# baseline (speedup 1.0000x reference)
"""Distributed flash-decoding attention kernel for 8 TRN2 NeuronCores.

B=1024 new tokens attend over a 32768-row KV cache plus the new block
(causal within the block). Sequence-parallel: each core handles 4224 keys
(4096 cache + 128 new), all 1024 queries.

Per key tile t (128 keys), single pass:
  scores s = kt_t.T @ qt          -> PSUM f32 [128k, 1024q]  (2 MMs of 512)
  e = exp(s)                      -> SBUF bf16 (ACT, batched (2+1)/3 tiles)
  pv += va_t.T @ e                -> PSUM f32 [128dv, 1024q] (2 MMs of 512)
  acc += e                        -> SBUF bf16 (DVE, softmax normalizer)
l = ones.T @ acc (PE partition reduce); partial [dv|l, q] blocks go to a
[1032, 128] DRAM tensor; ReduceScatter over q-blocks; epilogue transposes
the received [128dv, 128q] block and scales by 1/l.

PSUM: 6 banks score ring (3 tile slots x 2 banks) + 2 banks PV accum.
"""

import os
import sys

import numpy as np

for _p in ("/opt/trn_rl_repo",):
    if os.path.isdir(_p) and _p not in sys.path:
        sys.path.insert(0, _p)

import ml_dtypes  # noqa: E402
import concourse.bacc as bacc  # noqa: E402
import concourse.mybir as mybir  # noqa: E402
import concourse.tile as tile  # noqa: E402
from concourse.bass_utils import run_bass_kernel_spmd  # noqa: E402

N_CORES = 8
B, S, DK, DV = 1024, 32768, 128, 128
S_SH = S // N_CORES  # 4096 cache rows per core
B_SH = B // N_CORES  # 128 new rows per core
NKEY = S_SH + B_SH  # 4224 keys per core
NT = NKEY // 128  # 33 key tiles
RROW = DV + 1  # 129 rows per q-block in the reduce tensor (dv + l)
F32 = mybir.dt.float32
BF16 = mybir.dt.bfloat16
I32 = mybir.dt.int32

KT_CHUNKS = [(0, 1), (1, 5), (6, 13), (19, 14)]  # (first_tile, n)
VA_CHUNKS = [(0, 8), (8, 12), (20, 13)]


def _declare_io(nc):
    return dict(
        kt=nc.dram_tensor("kt", [128, NKEY], BF16, kind="ExternalInput"),
        qt=nc.dram_tensor("qt", [128, B], BF16, kind="ExternalInput"),
        va=nc.dram_tensor("va", [128, NKEY], BF16, kind="ExternalInput"),
        mask=nc.dram_tensor("mask", [128, B], BF16, kind="ExternalInput"),
        ident=nc.dram_tensor("ident", [128, 128], F32, kind="ExternalInput"),
        out=nc.dram_tensor("out", [B_SH, DV], F32, kind="ExternalOutput"),
    )


def _emit_body(nc, pools, io, part, stage=6, extras=None):
    """One pass of the compute body; writes the [1032, 128] partial to
    `part`. stage: 1=DMA only, 2=+scores, 3=+exp, 4=+PV, 5=+lacc,
    6=full (l reduce + copies + part DMA)."""
    p_in, p_e, p_acc, p_ep, ps_s, ps_pv = (
        pools["p_in"],
        pools["p_e"],
        pools["p_acc"],
        pools["p_ep"],
        pools["ps_s"],
        pools["ps_pv"],
    )

    # ---- input DMAs: kt/qt on the SP ring, va on the ACT ring.
    # Each chunk is its own pool tile so a score matmul only waits on the
    # one DMA that carries its kt tile (dep tracking is per-tile).
    qt_sbs = []
    for h in range(2):
        qh = p_in.tile([128, 512], BF16, name=f"qt{h}", tag=f"qt{h}")
        nc.sync.dma_start(qh[:], io["qt"][:, h * 512 : (h + 1) * 512])
        qt_sbs.append(qh)
        if h == 0:
            kt_sbs = []
            f, n = KT_CHUNKS[0]
            kc = p_in.tile([128, n * 128], BF16, name="kt0", tag="kt0")
            nc.sync.dma_start(kc[:], io["kt"][:, f * 128 : (f + n) * 128])
            kt_sbs.append(kc)
            va_sbs = []
            f, n = VA_CHUNKS[0]
            vc = p_in.tile([128, n * 128], BF16, name="va0", tag="va0")
            nc.scalar.dma_start(vc[:], io["va"][:, f * 128 : (f + n) * 128])
            va_sbs.append(vc)
    for i, (f, n) in enumerate(KT_CHUNKS[1:], 1):
        kc = p_in.tile([128, n * 128], BF16, name=f"kt{i}", tag=f"kt{i}")
        nc.sync.dma_start(kc[:], io["kt"][:, f * 128 : (f + n) * 128])
        kt_sbs.append(kc)
    for i, (f, n) in enumerate(VA_CHUNKS[1:], 1):
        vc = p_in.tile([128, n * 128], BF16, name=f"va{i}", tag=f"va{i}")
        nc.scalar.dma_start(vc[:], io["va"][:, f * 128 : (f + n) * 128])
        va_sbs.append(vc)
    mask01 = p_in.tile([128, B], BF16, name="mask01", tag="mask")
    nc.scalar.dma_start(mask01[:], io["mask"][:])

    def chunk_ap(chunks, sbs, t):
        for (f, n), tile_ in zip(chunks, sbs):
            if f <= t < f + n:
                return tile_[:, (t - f) * 128 : (t - f + 1) * 128]
        raise AssertionError(t)

    if stage < 2:
        return

    # score slots and e slots rotate through the pool (bufs=3 / bufs=9)
    pv_ps = ps_pv.tile([128, B], F32, name="pv_ps", tag="pv")
    accs = []
    if stage >= 5:
        acc0 = p_acc.tile([128, B], BF16, name="acc0", tag="acc")
        nc.vector.memset(acc0[:], 0.0)
        accs.append(acc0)
    e_tiles = {}

    def emit_scores(t):
        s_t = ps_s.tile([128, B], F32, name="s", tag="s")
        kt_ap = chunk_ap(KT_CHUNKS, kt_sbs, t)
        for h in range(2):
            nc.tensor.matmul(
                s_t[:, h * 512 : (h + 1) * 512],
                kt_ap,
                qt_sbs[h][:],
                start=True,
                stop=True,
            )
        if stage < 3:
            return
        e_t = p_e.tile([128, B], BF16, name="e", tag="e")
        e_tiles[t] = e_t
        nc.scalar.activation(
            e_t[:], s_t[:], mybir.ActivationFunctionType.Exp
        )

    def emit_pv(tr):
        e_ap = e_tiles[tr][:]
        if tr == NT - 1:
            em = p_e.tile([128, B], BF16, name="em", tag="em", bufs=1)
            nc.vector.tensor_tensor(
                out=em[:], in0=e_ap, in1=mask01[:], op=mybir.AluOpType.mult
            )
            e_ap = em[:]
        va_ap = chunk_ap(VA_CHUNKS, va_sbs, tr)
        for h in range(2):
            nc.tensor.matmul(
                pv_ps[:, h * 512 : (h + 1) * 512],
                va_ap,
                e_ap[:, h * 512 : (h + 1) * 512],
                start=(tr == 0),
                stop=(tr == NT - 1),
            )
        if stage >= 5:
            nxt = p_acc.tile([128, B], BF16, name="accn", tag="acc")
            nc.vector.tensor_tensor(
                out=nxt[:], in0=accs[-1][:], in1=e_ap, op=mybir.AluOpType.add
            )
            accs.append(nxt)

    # PV lags scores by TWO 3-tile groups (e pool bufs=9 gives the slack).
    LAG = 2
    n_groups = (NT + 2) // 3  # 11
    for g in range(n_groups + LAG):
        for j in range(3):
            t = 3 * g + j
            if t < NT:
                emit_scores(t)
        if stage >= 4 and g >= LAG:
            for j in range(3):
                tr = 3 * (g - LAG) + j
                if tr < NT:
                    emit_pv(tr)
    if stage < 6:
        return

    # ---- l = partition-reduce(acc) via ones-stationary matmul ----
    ones_sb = p_ep.tile([128, 1], BF16, name="ones_sb", tag="ones")
    nc.vector.memset(ones_sb[:], 1.0)
    l_ps = ps_s.tile([1, B], F32, name="l_ps", tag="s")
    for h in range(2):
        nc.tensor.matmul(
            l_ps[0:1, h * 512 : (h + 1) * 512],
            ones_sb[:],
            accs[-1][:, h * 512 : (h + 1) * 512],
            start=True,
            stop=True,
        )
    l_sb = p_ep.tile([1, B], F32, name="l_sb", tag="lsb")
    nc.vector.tensor_copy(l_sb[0:1, 0:512], l_ps[0:1, 0:512])
    nc.scalar.copy(l_sb[0:1, 512:1024], l_ps[0:1, 512:1024])
    if extras is not None:
        extras["acc"] = accs[-1]
        extras["l_sb"] = l_sb

    # ---- evacuate PV accum, DMA partial blocks ----
    pv_sb = p_ep.tile([128, B], F32, name="pv_sb", tag="pvsb")
    nc.vector.tensor_copy(pv_sb[:, 0:512], pv_ps[:, 0:512])
    nc.scalar.copy(pv_sb[:, 512:1024], pv_ps[:, 512:1024])

    part3 = part.rearrange("(j r) c -> j r c", r=RROW)
    nc.scalar.dma_start(
        part3[:, 0:DV, :].rearrange("j r c -> r j c"),
        pv_sb[:].rearrange("p (j c) -> p j c", j=8),
    )
    nc.sync.dma_start(
        part3[:, DV : DV + 1, :].rearrange("j o c -> o j c"),
        l_sb[0:1, :].rearrange("o (j c) -> o j c", j=8),
    )


def _emit_epilogue(nc, pools, io, red):
    p_ep, ps_s, ps_pv = pools["p_ep"], pools["ps_s"], pools["ps_pv"]
    ident = p_ep.tile([128, 128], F32, name="ident", tag="ident")
    nc.sync.dma_start(ident[:], io["ident"][:])
    red_dv = p_ep.tile([128, DV], F32, name="red_dv", tag="red_dv")
    nc.sync.dma_start(red_dv[:], red[0:DV, :])
    red_l = p_ep.tile([1, B_SH], F32, name="red_l", tag="red_l")
    nc.sync.dma_start(red_l[:], red[DV : DV + 1, :])
    linv = p_ep.tile([1, B_SH], F32, name="linv", tag="linv")
    nc.vector.reciprocal(linv[:], red_l[:])
    one1 = p_ep.tile([1, 1], F32, name="one1", tag="one1")
    nc.vector.memset(one1[:], 1.0)

    t_ps = ps_s.tile([128, B_SH], F32, name="t_ps", tag="s")
    nc.tensor.transpose(t_ps[:], red_dv[:], ident[:])
    lc_ps = ps_pv.tile([128, 1], F32, name="lc_ps", tag="pv")
    nc.tensor.matmul(lc_ps[:], linv[:], one1[:], start=True, stop=True)
    lc_sb = p_ep.tile([128, 1], F32, name="lc_sb", tag="lc_sb")
    nc.vector.tensor_copy(lc_sb[:], lc_ps[:])
    out_sb = p_ep.tile([128, DV], F32, name="out_sb", tag="out_sb")
    nc.vector.tensor_scalar_mul(out_sb[:], t_ps[:], lc_sb[:])
    nc.sync.dma_start(io["out"][:], out_sb[:])


def build_nc(loop_iters: int | None = None, stage: int = 6):
    """loop_iters=None: real kernel (compute + ReduceScatter + epilogue).
    loop_iters=N: timing variant, compute body in tc.For_i (no
    collective -- collectives can't sit inside control flow)."""
    nc = bacc.Bacc(
        "TRN2", target_bir_lowering=False, debug=False, num_devices=N_CORES
    )
    io = _declare_io(nc)
    with tile.TileContext(nc) as tc:
        with (
            tc.tile_pool(name="p_in", bufs=1) as p_in,
            tc.tile_pool(name="p_e", bufs=9) as p_e,
            tc.tile_pool(name="p_acc", bufs=2) as p_acc,
            tc.tile_pool(name="pmisc", bufs=1) as pmisc,
            tc.tile_pool(name="p_ep", bufs=1) as p_ep,
            tc.tile_pool(name="ps_s", bufs=3, space="PSUM") as ps_s,
            tc.tile_pool(name="ps_pv", bufs=1, space="PSUM") as ps_pv,
            tc.tile_pool(name="pdram", bufs=1, space="DRAM") as pdram,
        ):
            pools = dict(
                p_in=p_in, p_e=p_e, p_acc=p_acc, p_ep=p_ep, ps_s=ps_s,
                ps_pv=ps_pv, tc=tc,
            )
            # ACT table prewarm: tiny exp before any real dependency
            warm = pmisc.tile([128, 1], F32, name="warm", tag="warm")
            nc.vector.memset(warm[:], 0.0)
            warm_o = pmisc.tile([128, 1], BF16, name="warm_o", tag="warm_o")
            nc.scalar.activation(
                warm_o[:], warm[:], mybir.ActivationFunctionType.Exp
            )
            if loop_iters is None:
                part = pdram.tile([8 * RROW, B_SH], F32, name="part", tag="pa")
                red = pdram.tile([RROW, B_SH], F32, name="red", tag="re")
                _emit_body(nc, pools, io, part)
                nc.gpsimd.collective_compute(
                    "ReduceScatter",
                    mybir.AluOpType.add,
                    replica_groups=[list(range(N_CORES))],
                    ins=[part.opt()],
                    outs=[red.opt()],
                )
                _emit_epilogue(nc, pools, io, red)
            elif loop_iters == 0:
                # single body pass, no collective (for TimelineSim)
                part = pdram.tile([8 * RROW, B_SH], F32, name="part", tag="pa")
                _emit_body(nc, pools, io, part, stage=stage)
                out_sb = p_ep.tile([B_SH, DV], F32, name="out_sb1", tag="o0")
                nc.vector.memset(out_sb[:], 0.0)
                nc.sync.dma_start(io["out"][:], out_sb[:])
            else:
                part = pdram.tile([8 * RROW, B_SH], F32, name="part", tag="pa")
                with tc.For_i(0, max(loop_iters, 1), 1):
                    _emit_body(nc, pools, io, part, stage=stage)
                out_sb = p_ep.tile([B_SH, DV], F32, name="out_sb0", tag="o0")
                nc.vector.memset(out_sb[:], 0.0)
                nc.sync.dma_start(io["out"][:], out_sb[:])
    nc.compile()
    return nc


_CACHE: dict = {}


def _get_nc():
    if "nc" not in _CACHE:
        _CACHE["nc"] = build_nc()
    return _CACHE["nc"]


def make_in_maps(q, k, v, K_cache, V_cache):
    q = np.asarray(q, np.float32)
    k = np.asarray(k, np.float32)
    v = np.asarray(v, np.float32)
    K_cache = np.asarray(K_cache, np.float32)
    V_cache = np.asarray(V_cache, np.float32)

    scale = 1.0 / np.sqrt(np.float32(DK))
    qt = np.ascontiguousarray((q * scale).T).astype(ml_dtypes.bfloat16)

    in_maps = []
    for c in range(N_CORES):
        Ksh = np.concatenate(
            [K_cache[c * S_SH : (c + 1) * S_SH], k[c * B_SH : (c + 1) * B_SH]],
            axis=0,
        )  # [4224, 128]
        kt = np.ascontiguousarray(Ksh.T).astype(ml_dtypes.bfloat16)
        Vsh = np.concatenate(
            [V_cache[c * S_SH : (c + 1) * S_SH], v[c * B_SH : (c + 1) * B_SH]],
            axis=0,
        )  # [4224, 128]
        # va[p, t*128 + d] = V[t*128 + p, d]  (PE stationary layout)
        va = np.ascontiguousarray(
            Vsh.reshape(NT, 128, DV).transpose(1, 0, 2).reshape(128, NKEY)
        ).astype(ml_dtypes.bfloat16)
        thr = c * B_SH + np.arange(128, dtype=np.float32)
        mask = (
            np.arange(B, dtype=np.float32)[None, :] >= thr[:, None]
        ).astype(ml_dtypes.bfloat16)
        ident = np.eye(128, dtype=np.float32)
        in_maps.append(
            {"kt": kt, "qt": qt, "va": va, "mask": mask, "ident": ident}
        )
    return in_maps


def kernel(q, k, v, K_cache, V_cache):
    in_maps = make_in_maps(q, k, v, K_cache, V_cache)
    res = run_bass_kernel_spmd(
        _get_nc(), in_maps, core_ids=list(range(N_CORES))
    )
    out = np.concatenate(
        [res.results[c]["out"] for c in range(N_CORES)], axis=0
    )
    return np.ascontiguousarray(out, dtype=np.float32)


# revision 20
# speedup vs baseline: 1.9208x; 1.9208x over previous
"""Distributed flash-decoding attention kernel for 8 TRN2 NeuronCores.

B=1024 new tokens attend over a 32768-row KV cache plus the new block
(causal within the block). Sequence-parallel: each core handles 4224 keys
(4096 cache + 128 new), all 1024 queries.

Per key tile t (128 keys), single pass:
  scores s = kt_t.T @ qt          -> PSUM f32 [128k, 1024q]  (2 MMs of 512)
  e = exp(s)                      -> SBUF bf16 (ACT, batched (2+1)/3 tiles)
  pv += va_t.T @ e                -> PSUM f32 [128dv, 1024q] (2 MMs of 512)
  acc += e                        -> SBUF bf16 (DVE, softmax normalizer)
l = ones.T @ acc (PE partition reduce); partial [dv|l, q] blocks go to a
[1032, 128] DRAM tensor; ReduceScatter over q-blocks; epilogue transposes
the received [128dv, 128q] block and scales by 1/l.

PSUM: 6 banks score ring (3 tile slots x 2 banks) + 2 banks PV accum.
"""

import os
import sys

import numpy as np

for _p in ("/opt/trn_rl_repo",):
    if os.path.isdir(_p) and _p not in sys.path:
        sys.path.insert(0, _p)

import ml_dtypes  # noqa: E402
import concourse.bacc as bacc  # noqa: E402
import concourse.mybir as mybir  # noqa: E402
import concourse.tile as tile  # noqa: E402
from concourse.bass_utils import run_bass_kernel_spmd  # noqa: E402

N_CORES = 8
B, S, DK, DV = 1024, 32768, 128, 128
S_SH = S // N_CORES  # 4096 cache rows per core
B_SH = B // N_CORES  # 128 new rows per core
NKEY = S_SH + B_SH  # 4224 keys per core
NT = NKEY // 128  # 33 key tiles
RROW = DV + 1  # 129 rows per q-block in the reduce tensor (dv + l)
SPLIT_T = 18  # PV chunk split: tiles [0,18) -> part A, [18,33) -> part B
F32 = mybir.dt.float32
BF16 = mybir.dt.bfloat16
I32 = mybir.dt.int32

KT_CHUNKS = [(0, 1), (1, 5), (6, 13), (19, 14)]  # (first_tile, n)
VA_CHUNKS = [(0, 8), (8, 12), (20, 13)]


def _declare_io(nc):
    return dict(
        kt=nc.dram_tensor("kt", [128, NKEY], BF16, kind="ExternalInput"),
        qt=nc.dram_tensor("qt", [128, B], BF16, kind="ExternalInput"),
        va=nc.dram_tensor("va", [128, NKEY], BF16, kind="ExternalInput"),
        mask=nc.dram_tensor("mask", [128, B], BF16, kind="ExternalInput"),
        ident=nc.dram_tensor("ident", [128, 128], BF16, kind="ExternalInput"),
        out=nc.dram_tensor("out", [B_SH, DV], F32, kind="ExternalOutput"),
    )


def _emit_body(nc, pools, io, part, stage=6, extras=None):
    """One pass of the compute body; writes the [1032, 128] partial to
    `part`. stage: 1=DMA only, 2=+scores, 3=+exp, 4=+PV, 5=+lacc,
    6=full (l reduce + copies + part DMA)."""
    p_in, p_e, p_acc, p_ep, ps_s, ps_pv = (
        pools["p_in"],
        pools["p_e"],
        pools["p_acc"],
        pools["p_ep"],
        pools["ps_s"],
        pools["ps_pv"],
    )

    # ---- input DMAs: kt/qt on the SP ring, va on the ACT ring.
    # Each chunk is its own pool tile so a score matmul only waits on the
    # one DMA that carries its kt tile (dep tracking is per-tile).
    qt_sbs = []
    for h in range(2):
        qh = p_in.tile([128, 512], BF16, name=f"qt{h}", tag=f"qt{h}")
        nc.sync.dma_start(qh[:], io["qt"][:, h * 512 : (h + 1) * 512])
        qt_sbs.append(qh)
        if h == 0:
            kt_sbs = []
            f, n = KT_CHUNKS[0]
            kc = p_in.tile([128, n * 128], BF16, name="kt0", tag="kt0")
            nc.sync.dma_start(kc[:], io["kt"][:, f * 128 : (f + n) * 128])
            kt_sbs.append(kc)
            va_sbs = []
            f, n = VA_CHUNKS[0]
            vc = p_in.tile([128, n * 128], BF16, name="va0", tag="va0")
            nc.scalar.dma_start(vc[:], io["va"][:, f * 128 : (f + n) * 128])
            va_sbs.append(vc)
    for i, (f, n) in enumerate(KT_CHUNKS[1:], 1):
        kc = p_in.tile([128, n * 128], BF16, name=f"kt{i}", tag=f"kt{i}")
        nc.sync.dma_start(kc[:], io["kt"][:, f * 128 : (f + n) * 128])
        kt_sbs.append(kc)
    for i, (f, n) in enumerate(VA_CHUNKS[1:], 1):
        vc = p_in.tile([128, n * 128], BF16, name=f"va{i}", tag=f"va{i}")
        nc.scalar.dma_start(vc[:], io["va"][:, f * 128 : (f + n) * 128])
        va_sbs.append(vc)
    mask01 = p_in.tile([128, B], BF16, name="mask01", tag="mask")
    nc.scalar.dma_start(mask01[:], io["mask"][:])

    def chunk_ap(chunks, sbs, t):
        for (f, n), tile_ in zip(chunks, sbs):
            if f <= t < f + n:
                return tile_[:, (t - f) * 128 : (t - f + 1) * 128]
        raise AssertionError(t)

    if stage < 2:
        return

    # score slots and e slots rotate through the pool (bufs=3 / bufs=9)
    pv_ps = ps_pv.tile([128, B], F32, name="pv_ps", tag="pv")
    accs = []
    if stage >= 5:
        acc0 = p_acc.tile([128, B], BF16, name="acc0", tag="acc")
        nc.vector.memset(acc0[:], 0.0)
        accs.append(acc0)
    e_tiles = {}

    def emit_scores(t):
        s_t = ps_s.tile([128, B], F32, name="s", tag="s")
        kt_ap = chunk_ap(KT_CHUNKS, kt_sbs, t)
        for h in range(2):
            nc.tensor.matmul(
                s_t[:, h * 512 : (h + 1) * 512],
                kt_ap,
                qt_sbs[h][:],
                start=True,
                stop=True,
            )
        if stage < 3:
            return
        e_t = p_e.tile([128, B], BF16, name="e", tag="e")
        e_tiles[t] = e_t
        nc.scalar.activation(
            e_t[:], s_t[:], mybir.ActivationFunctionType.Exp
        )

    def emit_pv(tr):
        e_ap = e_tiles[tr][:]
        if tr == NT - 1:
            em = p_e.tile([128, B], BF16, name="em", tag="em", bufs=1)
            nc.vector.tensor_tensor(
                out=em[:], in0=e_ap, in1=mask01[:], op=mybir.AluOpType.mult
            )
            e_ap = em[:]
        va_ap = chunk_ap(VA_CHUNKS, va_sbs, tr)
        for h in range(2):
            nc.tensor.matmul(
                pv_ps[:, h * 512 : (h + 1) * 512],
                va_ap,
                e_ap[:, h * 512 : (h + 1) * 512],
                start=(tr == 0 or tr == SPLIT_T),
                stop=(tr == SPLIT_T - 1 or tr == NT - 1),
            )
        if stage >= 5:
            nxt = p_acc.tile([128, B], BF16, name="accn", tag="acc")
            nc.vector.tensor_tensor(
                out=nxt[:], in0=accs[-1][:], in1=e_ap, op=mybir.AluOpType.add
            )
            accs.append(nxt)

    # PV lags scores by TWO 3-tile groups (e pool bufs=9 gives the slack).
    ones_sb = p_ep.tile([128, 1], BF16, name="ones_sb", tag="ones")
    nc.vector.memset(ones_sb[:], 1.0)

    def finalize_chunk(part_t, suffix, evac_eng):
        """l partition-reduce + PV evacuation + partial DMA for one chunk.
        evac_eng: engine for the second-half copies ('dve' keeps the ACT
        queue clean mid-stream; 'act' parallelizes at the end)."""
        l_ps = ps_s.tile([1, B], F32, name=f"l_ps{suffix}", tag="s")
        for h in range(2):
            nc.tensor.matmul(
                l_ps[0:1, h * 512 : (h + 1) * 512],
                ones_sb[:],
                accs[-1][:, h * 512 : (h + 1) * 512],
                start=True,
                stop=True,
            )
        l_sb = p_ep.tile([1, B], BF16, name=f"l_sb{suffix}", tag=f"l{suffix}")
        pv_sb = p_ep.tile(
            [128, B], BF16, name=f"pv_sb{suffix}", tag=f"pv{suffix}"
        )
        nc.vector.tensor_copy(l_sb[0:1, 0:512], l_ps[0:1, 0:512])
        nc.vector.tensor_copy(pv_sb[:, 0:512], pv_ps[:, 0:512])
        if evac_eng == "act":
            nc.scalar.copy(l_sb[0:1, 512:1024], l_ps[0:1, 512:1024])
            nc.scalar.copy(pv_sb[:, 512:1024], pv_ps[:, 512:1024])
        else:
            nc.vector.tensor_copy(l_sb[0:1, 512:1024], l_ps[0:1, 512:1024])
            nc.vector.tensor_copy(pv_sb[:, 512:1024], pv_ps[:, 512:1024])
        part3 = part_t.rearrange("(j r) c -> j r c", r=RROW)
        nc.sync.dma_start(
            part3[:, 0:DV, :].rearrange("j r c -> r j c"),
            pv_sb[:].rearrange("p (j c) -> p j c", j=8),
        )
        nc.sync.dma_start(
            part3[:, DV : DV + 1, :].rearrange("j o c -> o j c"),
            l_sb[0:1, :].rearrange("o (j c) -> o j c", j=8),
        )

    part_a, part_b = part
    LAG = 2
    n_groups = (NT + 2) // 3  # 11
    for g in range(n_groups + LAG):
        for j in range(3):
            t = 3 * g + j
            if t < NT:
                emit_scores(t)
        if stage >= 4 and g >= LAG:
            for j in range(3):
                tr = 3 * (g - LAG) + j
                if tr < NT:
                    emit_pv(tr)
                    if stage >= 6 and tr == SPLIT_T - 1:
                        finalize_chunk(part_a, "a", "dve")
                        if stage >= 5:
                            accB = p_acc.tile(
                                [128, B], BF16, name="accB", tag="acc"
                            )
                            nc.vector.memset(accB[:], 0.0)
                            accs.append(accB)
    if stage < 6:
        return
    finalize_chunk(part_b, "b", "act")


def _emit_epilogue(nc, pools, io, red_a, red_b):
    p_ep, ps_s, ps_pv = pools["p_ep"], pools["ps_s"], pools["ps_pv"]
    ident = p_ep.tile([128, 128], BF16, name="ident", tag="ident")
    nc.sync.dma_start(ident[:], io["ident"][:])
    ra_dv = p_ep.tile([DV, B_SH], BF16, name="ra_dv", tag="ra_dv")
    nc.sync.dma_start(ra_dv[:], red_a[0:DV, :])
    ra_l = p_ep.tile([1, B_SH], BF16, name="ra_l", tag="ra_l")
    nc.sync.dma_start(ra_l[:], red_a[DV : DV + 1, :])
    rb_dv = p_ep.tile([DV, B_SH], BF16, name="rb_dv", tag="rb_dv")
    nc.sync.dma_start(rb_dv[:], red_b[0:DV, :])
    rb_l = p_ep.tile([1, B_SH], BF16, name="rb_l", tag="rb_l")
    nc.sync.dma_start(rb_l[:], red_b[DV : DV + 1, :])
    red_dv = p_ep.tile([128, B_SH], BF16, name="red_dv", tag="red_dv")
    nc.vector.tensor_tensor(
        out=red_dv[:], in0=ra_dv[:], in1=rb_dv[:], op=mybir.AluOpType.add
    )
    red_l = p_ep.tile([1, B_SH], F32, name="red_l", tag="red_l")
    nc.vector.tensor_tensor(
        out=red_l[:], in0=ra_l[:], in1=rb_l[:], op=mybir.AluOpType.add
    )
    linv = p_ep.tile([1, B_SH], F32, name="linv", tag="linv")
    nc.vector.reciprocal(linv[:], red_l[:])
    one1 = p_ep.tile([1, 1], F32, name="one1", tag="one1")
    nc.vector.memset(one1[:], 1.0)

    t_ps = ps_s.tile([128, B_SH], BF16, name="t_ps", tag="s")
    nc.tensor.transpose(t_ps[:], red_dv[:], ident[:])
    lc_ps = ps_pv.tile([128, 1], F32, name="lc_ps", tag="pv")
    nc.tensor.matmul(lc_ps[:], linv[:], one1[:], start=True, stop=True)
    lc_sb = p_ep.tile([128, 1], F32, name="lc_sb", tag="lc_sb")
    nc.vector.tensor_copy(lc_sb[:], lc_ps[:])
    out_sb = p_ep.tile([128, DV], F32, name="out_sb", tag="out_sb")
    nc.vector.tensor_scalar_mul(out_sb[:], t_ps[:], lc_sb[:])
    nc.sync.dma_start(io["out"][:], out_sb[:])


def build_nc(loop_iters: int | None = None, stage: int = 6):
    """loop_iters=None: real kernel (compute + ReduceScatter + epilogue).
    loop_iters=N: timing variant, compute body in tc.For_i (no
    collective -- collectives can't sit inside control flow)."""
    nc = bacc.Bacc(
        "TRN2", target_bir_lowering=False, debug=False, num_devices=N_CORES
    )
    io = _declare_io(nc)
    with tile.TileContext(nc) as tc:
        with (
            tc.tile_pool(name="p_in", bufs=1) as p_in,
            tc.tile_pool(name="p_e", bufs=9) as p_e,
            tc.tile_pool(name="p_acc", bufs=2) as p_acc,
            tc.tile_pool(name="pmisc", bufs=1) as pmisc,
            tc.tile_pool(name="p_ep", bufs=1) as p_ep,
            tc.tile_pool(name="ps_s", bufs=3, space="PSUM") as ps_s,
            tc.tile_pool(name="ps_pv", bufs=1, space="PSUM") as ps_pv,
            tc.tile_pool(name="pdram", bufs=1, space="DRAM") as pdram,
        ):
            pools = dict(
                p_in=p_in, p_e=p_e, p_acc=p_acc, p_ep=p_ep, ps_s=ps_s,
                ps_pv=ps_pv, tc=tc,
            )
            # ACT table prewarm: tiny exp before any real dependency
            warm = pmisc.tile([128, 1], F32, name="warm", tag="warm")
            nc.vector.memset(warm[:], 0.0)
            warm_o = pmisc.tile([128, 1], BF16, name="warm_o", tag="warm_o")
            nc.scalar.activation(
                warm_o[:], warm[:], mybir.ActivationFunctionType.Exp
            )
            part_a = pdram.tile(
                [8 * RROW, B_SH], BF16, name="part_a", tag="pa"
            )
            part_b = pdram.tile(
                [8 * RROW, B_SH], BF16, name="part_b", tag="pb"
            )
            part = (part_a, part_b)
            if loop_iters is None:
                red_a = pdram.tile([RROW, B_SH], BF16, name="red_a", tag="ra")
                red_b = pdram.tile([RROW, B_SH], BF16, name="red_b", tag="rb")
                _emit_body(nc, pools, io, part)
                for pt, rt in ((part_a, red_a), (part_b, red_b)):
                    nc.gpsimd.collective_compute(
                        "ReduceScatter",
                        mybir.AluOpType.add,
                        replica_groups=[list(range(N_CORES))],
                        ins=[pt.opt()],
                        outs=[rt.opt()],
                    )
                _emit_epilogue(nc, pools, io, red_a, red_b)
            elif loop_iters == 0:
                # single body pass, no collective (for TimelineSim)
                _emit_body(nc, pools, io, part, stage=stage)
                out_sb = p_ep.tile([B_SH, DV], F32, name="out_sb1", tag="o0")
                nc.vector.memset(out_sb[:], 0.0)
                nc.sync.dma_start(io["out"][:], out_sb[:])
            else:
                with tc.For_i(0, max(loop_iters, 1), 1):
                    _emit_body(nc, pools, io, part, stage=stage)
                out_sb = p_ep.tile([B_SH, DV], F32, name="out_sb0", tag="o0")
                nc.vector.memset(out_sb[:], 0.0)
                nc.sync.dma_start(io["out"][:], out_sb[:])
    nc.compile()
    return nc


_CACHE: dict = {}


def _get_nc():
    if "nc" not in _CACHE:
        _CACHE["nc"] = build_nc()
    return _CACHE["nc"]


def make_in_maps(q, k, v, K_cache, V_cache):
    q = np.asarray(q, np.float32)
    k = np.asarray(k, np.float32)
    v = np.asarray(v, np.float32)
    K_cache = np.asarray(K_cache, np.float32)
    V_cache = np.asarray(V_cache, np.float32)

    scale = 1.0 / np.sqrt(np.float32(DK))
    qt = np.ascontiguousarray((q * scale).T).astype(ml_dtypes.bfloat16)

    in_maps = []
    for c in range(N_CORES):
        Ksh = np.concatenate(
            [K_cache[c * S_SH : (c + 1) * S_SH], k[c * B_SH : (c + 1) * B_SH]],
            axis=0,
        )  # [4224, 128]
        kt = np.ascontiguousarray(Ksh.T).astype(ml_dtypes.bfloat16)
        Vsh = np.concatenate(
            [V_cache[c * S_SH : (c + 1) * S_SH], v[c * B_SH : (c + 1) * B_SH]],
            axis=0,
        )  # [4224, 128]
        # va[p, t*128 + d] = V[t*128 + p, d]  (PE stationary layout)
        va = np.ascontiguousarray(
            Vsh.reshape(NT, 128, DV).transpose(1, 0, 2).reshape(128, NKEY)
        ).astype(ml_dtypes.bfloat16)
        thr = c * B_SH + np.arange(128, dtype=np.float32)
        mask = (
            np.arange(B, dtype=np.float32)[None, :] >= thr[:, None]
        ).astype(ml_dtypes.bfloat16)
        ident = np.eye(128, dtype=ml_dtypes.bfloat16)
        in_maps.append(
            {"kt": kt, "qt": qt, "va": va, "mask": mask, "ident": ident}
        )
    return in_maps


def kernel(q, k, v, K_cache, V_cache):
    in_maps = make_in_maps(q, k, v, K_cache, V_cache)
    res = run_bass_kernel_spmd(
        _get_nc(), in_maps, core_ids=list(range(N_CORES))
    )
    out = np.concatenate(
        [res.results[c]["out"] for c in range(N_CORES)], axis=0
    )
    return np.ascontiguousarray(out, dtype=np.float32)


# revision 21
# speedup vs baseline: 7.7359x; 4.0275x over previous
"""Distributed flash-decoding attention kernel for 8 TRN2 NeuronCores.

B=1024 new tokens attend over a 32768-row KV cache plus the new block
(causal within the block). Sequence-parallel: each core handles 4224 keys
(4096 cache + 128 new), all 1024 queries.

Per key tile t (128 keys), single pass:
  scores s = kt_t.T @ qt          -> PSUM f32 [128k, 1024q]  (2 MMs of 512)
  e = exp(s)                      -> SBUF bf16 (ACT, batched (2+1)/3 tiles)
  pv += va_t.T @ e                -> PSUM f32 [128dv, 1024q] (2 MMs of 512)
  acc += e                        -> SBUF bf16 (DVE, softmax normalizer)
l = ones.T @ acc (PE partition reduce); partial [dv|l, q] blocks go to a
[1032, 128] DRAM tensor; ReduceScatter over q-blocks; epilogue transposes
the received [128dv, 128q] block and scales by 1/l.

PSUM: 6 banks score ring (3 tile slots x 2 banks) + 2 banks PV accum.
"""

import os
import sys

import numpy as np

for _p in ("/opt/trn_rl_repo",):
    if os.path.isdir(_p) and _p not in sys.path:
        sys.path.insert(0, _p)

import ml_dtypes  # noqa: E402
import concourse.bacc as bacc  # noqa: E402
import concourse.mybir as mybir  # noqa: E402
import concourse.tile as tile  # noqa: E402
from concourse.bass_utils import run_bass_kernel_spmd  # noqa: E402

N_CORES = 8
B, S, DK, DV = 1024, 32768, 128, 128
S_SH = S // N_CORES  # 4096 cache rows per core
B_SH = B // N_CORES  # 128 new rows per core
NKEY = S_SH + B_SH  # 4224 keys per core
NT = NKEY // 128  # 33 key tiles
RROW = DV + 1  # 129 rows per q-block in the reduce tensor (dv + l)
SPLIT_T = 18  # PV chunk split: tiles [0,18) -> part A, [18,33) -> part B
F32 = mybir.dt.float32
BF16 = mybir.dt.bfloat16
I32 = mybir.dt.int32

KT_CHUNKS = [(0, 1), (1, 5), (6, 13), (19, 14)]  # (first_tile, n)
VA_CHUNKS = [(0, 8), (8, 12), (20, 13)]


def _declare_io(nc):
    return dict(
        kt=nc.dram_tensor("kt", [128, NKEY], BF16, kind="ExternalInput"),
        qt=nc.dram_tensor("qt", [128, B], BF16, kind="ExternalInput"),
        va=nc.dram_tensor("va", [128, NKEY], BF16, kind="ExternalInput"),
        mask=nc.dram_tensor("mask", [128, B], BF16, kind="ExternalInput"),
        ident=nc.dram_tensor("ident", [128, 128], BF16, kind="ExternalInput"),
        out=nc.dram_tensor("out", [B_SH, DV], F32, kind="ExternalOutput"),
    )


def _emit_body(nc, pools, io, part, stage=6, extras=None):
    """One pass of the compute body; writes the [1032, 128] partial to
    `part`. stage: 1=DMA only, 2=+scores, 3=+exp, 4=+PV, 5=+lacc,
    6=full (l reduce + copies + part DMA)."""
    p_in, p_e, p_acc, p_ep, ps_s, ps_pv = (
        pools["p_in"],
        pools["p_e"],
        pools["p_acc"],
        pools["p_ep"],
        pools["ps_s"],
        pools["ps_pv"],
    )

    # ---- input DMAs: ALL on the SP ring so the ACT queue carries only
    # exp instructions (each HWDGE trigger costs ~0.6-1.3us of queue time).
    # kt tile 0 first so the first score matmul starts ASAP.
    kt_sbs = []
    f, n = KT_CHUNKS[0]
    kc = p_in.tile([128, n * 128], BF16, name="kt0", tag="kt0")
    nc.sync.dma_start(kc[:], io["kt"][:, f * 128 : (f + n) * 128])
    kt_sbs.append(kc)
    qt_sbs = []
    for h in range(2):
        qh = p_in.tile([128, 512], BF16, name=f"qt{h}", tag=f"qt{h}")
        nc.sync.dma_start(qh[:], io["qt"][:, h * 512 : (h + 1) * 512])
        qt_sbs.append(qh)
    va_sbs = []
    f, n = VA_CHUNKS[0]
    vc = p_in.tile([128, n * 128], BF16, name="va0", tag="va0")
    nc.sync.dma_start(vc[:], io["va"][:, f * 128 : (f + n) * 128])
    va_sbs.append(vc)
    for i, (f, n) in enumerate(KT_CHUNKS[1:], 1):
        kc = p_in.tile([128, n * 128], BF16, name=f"kt{i}", tag=f"kt{i}")
        nc.sync.dma_start(kc[:], io["kt"][:, f * 128 : (f + n) * 128])
        kt_sbs.append(kc)
    for i, (f, n) in enumerate(VA_CHUNKS[1:], 1):
        vc = p_in.tile([128, n * 128], BF16, name=f"va{i}", tag=f"va{i}")
        nc.sync.dma_start(vc[:], io["va"][:, f * 128 : (f + n) * 128])
        va_sbs.append(vc)
    mask01 = p_in.tile([128, B], BF16, name="mask01", tag="mask")
    nc.sync.dma_start(mask01[:], io["mask"][:])

    def chunk_ap(chunks, sbs, t):
        for (f, n), tile_ in zip(chunks, sbs):
            if f <= t < f + n:
                return tile_[:, (t - f) * 128 : (t - f + 1) * 128]
        raise AssertionError(t)

    if stage < 2:
        return

    # score slots and e slots rotate through the pool (bufs=3 / bufs=9)
    pv_ps = ps_pv.tile([128, B], F32, name="pv_ps", tag="pv")
    accs = []
    if stage >= 5:
        acc0 = p_acc.tile([128, B], BF16, name="acc0", tag="acc")
        nc.vector.memset(acc0[:], 0.0)
        accs.append(acc0)
    e_tiles = {}

    def emit_scores(t):
        s_t = ps_s.tile([128, B], F32, name="s", tag="s")
        kt_ap = chunk_ap(KT_CHUNKS, kt_sbs, t)
        for h in range(2):
            nc.tensor.matmul(
                s_t[:, h * 512 : (h + 1) * 512],
                kt_ap,
                qt_sbs[h][:],
                start=True,
                stop=True,
            )
        if stage < 3:
            return
        e_t = p_e.tile([128, B], BF16, name="e", tag="e")
        e_tiles[t] = e_t
        nc.scalar.activation(
            e_t[:], s_t[:], mybir.ActivationFunctionType.Exp
        )

    def emit_pv(tr):
        e_ap = e_tiles[tr][:]
        if tr == NT - 1:
            em = p_e.tile([128, B], BF16, name="em", tag="em", bufs=1)
            nc.vector.tensor_tensor(
                out=em[:], in0=e_ap, in1=mask01[:], op=mybir.AluOpType.mult
            )
            e_ap = em[:]
        va_ap = chunk_ap(VA_CHUNKS, va_sbs, tr)
        for h in range(2):
            nc.tensor.matmul(
                pv_ps[:, h * 512 : (h + 1) * 512],
                va_ap,
                e_ap[:, h * 512 : (h + 1) * 512],
                start=(tr == 0 or tr == SPLIT_T),
                stop=(tr == SPLIT_T - 1 or tr == NT - 1),
            )
        if stage >= 5:
            nxt = p_acc.tile([128, B], BF16, name="accn", tag="acc")
            nc.vector.tensor_tensor(
                out=nxt[:], in0=accs[-1][:], in1=e_ap, op=mybir.AluOpType.add
            )
            accs.append(nxt)

    # PV lags scores by TWO 3-tile groups (e pool bufs=9 gives the slack).
    ones_sb = p_ep.tile([128, 1], BF16, name="ones_sb", tag="ones")
    nc.vector.memset(ones_sb[:], 1.0)

    def finalize_chunk(part_t, suffix, evac_eng):
        """l partition-reduce + PV evacuation + partial DMA for one chunk.
        evac_eng: engine for the second-half copies ('dve' keeps the ACT
        queue clean mid-stream; 'act' parallelizes at the end)."""
        l_ps = ps_s.tile([1, B], F32, name=f"l_ps{suffix}", tag="s")
        for h in range(2):
            nc.tensor.matmul(
                l_ps[0:1, h * 512 : (h + 1) * 512],
                ones_sb[:],
                accs[-1][:, h * 512 : (h + 1) * 512],
                start=True,
                stop=True,
            )
        l_sb = p_ep.tile([1, B], BF16, name=f"l_sb{suffix}", tag=f"l{suffix}")
        pv_sb = p_ep.tile(
            [128, B], BF16, name=f"pv_sb{suffix}", tag=f"pv{suffix}"
        )
        nc.vector.tensor_copy(l_sb[0:1, 0:512], l_ps[0:1, 0:512])
        nc.vector.tensor_copy(pv_sb[:, 0:512], pv_ps[:, 0:512])
        if evac_eng == "act":
            nc.scalar.copy(l_sb[0:1, 512:1024], l_ps[0:1, 512:1024])
            nc.scalar.copy(pv_sb[:, 512:1024], pv_ps[:, 512:1024])
        else:
            nc.vector.tensor_copy(l_sb[0:1, 512:1024], l_ps[0:1, 512:1024])
            nc.vector.tensor_copy(pv_sb[:, 512:1024], pv_ps[:, 512:1024])
        part3 = part_t.rearrange("(j r) c -> j r c", r=RROW)
        nc.sync.dma_start(
            part3[:, 0:DV, :].rearrange("j r c -> r j c"),
            pv_sb[:].rearrange("p (j c) -> p j c", j=8),
        )
        nc.sync.dma_start(
            part3[:, DV : DV + 1, :].rearrange("j o c -> o j c"),
            l_sb[0:1, :].rearrange("o (j c) -> o j c", j=8),
        )

    part_a, part_b = part
    LAG = 2
    n_groups = (NT + 2) // 3  # 11
    for g in range(n_groups + LAG):
        for j in range(3):
            t = 3 * g + j
            if t < NT:
                emit_scores(t)
        if stage >= 4 and g >= LAG:
            for j in range(3):
                tr = 3 * (g - LAG) + j
                if tr < NT:
                    emit_pv(tr)
                    if stage >= 6 and tr == SPLIT_T - 1:
                        finalize_chunk(part_a, "a", "dve")
                        if stage >= 5:
                            accB = p_acc.tile(
                                [128, B], BF16, name="accB", tag="acc"
                            )
                            nc.vector.memset(accB[:], 0.0)
                            accs.append(accB)
    if stage < 6:
        return
    finalize_chunk(part_b, "b", "act")


def _emit_epilogue(nc, pools, io, red_a, red_b):
    p_ep, ps_s, ps_pv = pools["p_ep"], pools["ps_s"], pools["ps_pv"]
    ident = p_ep.tile([128, 128], BF16, name="ident", tag="ident")
    nc.sync.dma_start(ident[:], io["ident"][:])
    ra_dv = p_ep.tile([DV, B_SH], BF16, name="ra_dv", tag="ra_dv")
    nc.sync.dma_start(ra_dv[:], red_a[0:DV, :])
    ra_l = p_ep.tile([1, B_SH], BF16, name="ra_l", tag="ra_l")
    nc.sync.dma_start(ra_l[:], red_a[DV : DV + 1, :])
    rb_dv = p_ep.tile([DV, B_SH], BF16, name="rb_dv", tag="rb_dv")
    nc.sync.dma_start(rb_dv[:], red_b[0:DV, :])
    rb_l = p_ep.tile([1, B_SH], BF16, name="rb_l", tag="rb_l")
    nc.sync.dma_start(rb_l[:], red_b[DV : DV + 1, :])
    red_dv = p_ep.tile([128, B_SH], BF16, name="red_dv", tag="red_dv")
    nc.vector.tensor_tensor(
        out=red_dv[:], in0=ra_dv[:], in1=rb_dv[:], op=mybir.AluOpType.add
    )
    red_l = p_ep.tile([1, B_SH], F32, name="red_l", tag="red_l")
    nc.vector.tensor_tensor(
        out=red_l[:], in0=ra_l[:], in1=rb_l[:], op=mybir.AluOpType.add
    )
    linv = p_ep.tile([1, B_SH], F32, name="linv", tag="linv")
    nc.vector.reciprocal(linv[:], red_l[:])
    one1 = p_ep.tile([1, 1], F32, name="one1", tag="one1")
    nc.vector.memset(one1[:], 1.0)

    t_ps = ps_s.tile([128, B_SH], BF16, name="t_ps", tag="s")
    nc.tensor.transpose(t_ps[:], red_dv[:], ident[:])
    lc_ps = ps_pv.tile([128, 1], F32, name="lc_ps", tag="pv")
    nc.tensor.matmul(lc_ps[:], linv[:], one1[:], start=True, stop=True)
    lc_sb = p_ep.tile([128, 1], F32, name="lc_sb", tag="lc_sb")
    nc.vector.tensor_copy(lc_sb[:], lc_ps[:])
    out_sb = p_ep.tile([128, DV], F32, name="out_sb", tag="out_sb")
    nc.vector.tensor_scalar_mul(out_sb[:], t_ps[:], lc_sb[:])
    nc.sync.dma_start(io["out"][:], out_sb[:])


def build_nc(loop_iters: int | None = None, stage: int = 6):
    """loop_iters=None: real kernel (compute + ReduceScatter + epilogue).
    loop_iters=N: timing variant, compute body in tc.For_i (no
    collective -- collectives can't sit inside control flow)."""
    nc = bacc.Bacc(
        "TRN2", target_bir_lowering=False, debug=False, num_devices=N_CORES
    )
    io = _declare_io(nc)
    with tile.TileContext(nc) as tc:
        with (
            tc.tile_pool(name="p_in", bufs=1) as p_in,
            tc.tile_pool(name="p_e", bufs=9) as p_e,
            tc.tile_pool(name="p_acc", bufs=2) as p_acc,
            tc.tile_pool(name="pmisc", bufs=1) as pmisc,
            tc.tile_pool(name="p_ep", bufs=1) as p_ep,
            tc.tile_pool(name="ps_s", bufs=3, space="PSUM") as ps_s,
            tc.tile_pool(name="ps_pv", bufs=1, space="PSUM") as ps_pv,
            tc.tile_pool(name="pdram", bufs=1, space="DRAM") as pdram,
        ):
            pools = dict(
                p_in=p_in, p_e=p_e, p_acc=p_acc, p_ep=p_ep, ps_s=ps_s,
                ps_pv=ps_pv, tc=tc,
            )
            # ACT table prewarm: tiny exp before any real dependency
            warm = pmisc.tile([128, 1], F32, name="warm", tag="warm")
            nc.vector.memset(warm[:], 0.0)
            warm_o = pmisc.tile([128, 1], BF16, name="warm_o", tag="warm_o")
            nc.scalar.activation(
                warm_o[:], warm[:], mybir.ActivationFunctionType.Exp
            )
            part_a = pdram.tile(
                [8 * RROW, B_SH], BF16, name="part_a", tag="pa"
            )
            part_b = pdram.tile(
                [8 * RROW, B_SH], BF16, name="part_b", tag="pb"
            )
            part = (part_a, part_b)
            if loop_iters is None:
                red_a = pdram.tile([RROW, B_SH], BF16, name="red_a", tag="ra")
                red_b = pdram.tile([RROW, B_SH], BF16, name="red_b", tag="rb")
                _emit_body(nc, pools, io, part)
                for pt, rt in ((part_a, red_a), (part_b, red_b)):
                    nc.gpsimd.collective_compute(
                        "ReduceScatter",
                        mybir.AluOpType.add,
                        replica_groups=[list(range(N_CORES))],
                        ins=[pt.opt()],
                        outs=[rt.opt()],
                    )
                _emit_epilogue(nc, pools, io, red_a, red_b)
            elif loop_iters == 0:
                # single body pass, no collective (for TimelineSim)
                _emit_body(nc, pools, io, part, stage=stage)
                out_sb = p_ep.tile([B_SH, DV], F32, name="out_sb1", tag="o0")
                nc.vector.memset(out_sb[:], 0.0)
                nc.sync.dma_start(io["out"][:], out_sb[:])
            else:
                with tc.For_i(0, max(loop_iters, 1), 1):
                    _emit_body(nc, pools, io, part, stage=stage)
                out_sb = p_ep.tile([B_SH, DV], F32, name="out_sb0", tag="o0")
                nc.vector.memset(out_sb[:], 0.0)
                nc.sync.dma_start(io["out"][:], out_sb[:])
    nc.compile()
    return nc


_CACHE: dict = {}


def _get_nc():
    if "nc" not in _CACHE:
        _CACHE["nc"] = build_nc()
    return _CACHE["nc"]


def make_in_maps(q, k, v, K_cache, V_cache):
    q = np.asarray(q, np.float32)
    k = np.asarray(k, np.float32)
    v = np.asarray(v, np.float32)
    K_cache = np.asarray(K_cache, np.float32)
    V_cache = np.asarray(V_cache, np.float32)

    scale = 1.0 / np.sqrt(np.float32(DK))
    qt = np.ascontiguousarray((q * scale).T).astype(ml_dtypes.bfloat16)

    in_maps = []
    for c in range(N_CORES):
        Ksh = np.concatenate(
            [K_cache[c * S_SH : (c + 1) * S_SH], k[c * B_SH : (c + 1) * B_SH]],
            axis=0,
        )  # [4224, 128]
        kt = np.ascontiguousarray(Ksh.T).astype(ml_dtypes.bfloat16)
        Vsh = np.concatenate(
            [V_cache[c * S_SH : (c + 1) * S_SH], v[c * B_SH : (c + 1) * B_SH]],
            axis=0,
        )  # [4224, 128]
        # va[p, t*128 + d] = V[t*128 + p, d]  (PE stationary layout)
        va = np.ascontiguousarray(
            Vsh.reshape(NT, 128, DV).transpose(1, 0, 2).reshape(128, NKEY)
        ).astype(ml_dtypes.bfloat16)
        thr = c * B_SH + np.arange(128, dtype=np.float32)
        mask = (
            np.arange(B, dtype=np.float32)[None, :] >= thr[:, None]
        ).astype(ml_dtypes.bfloat16)
        ident = np.eye(128, dtype=ml_dtypes.bfloat16)
        in_maps.append(
            {"kt": kt, "qt": qt, "va": va, "mask": mask, "ident": ident}
        )
    return in_maps


def kernel(q, k, v, K_cache, V_cache):
    in_maps = make_in_maps(q, k, v, K_cache, V_cache)
    res = run_bass_kernel_spmd(
        _get_nc(), in_maps, core_ids=list(range(N_CORES))
    )
    out = np.concatenate(
        [res.results[c]["out"] for c in range(N_CORES)], axis=0
    )
    return np.ascontiguousarray(out, dtype=np.float32)
